# revision 3
# baseline (speedup 1.0000x reference)
"""DistanceConv2 GNN edge-MLP kernel for Trainium2 (8 NeuronCores).

out[e] = W2 @ relu(W1 @ [x[src_e]; x[dst_e]; attr_e] + b1) + b2   for 800K edges.

Strategy (edges sharded 8-way data-parallel, x + weights replicated):
  - x is stored node-major in DRAM (bf16 [50000, 128], 256B rows).
  - Endpoint features are fetched with gpsimd.dma_gather(transpose=True):
    GPSIMD only generates descriptors, the DMA engines move the data and
    write it feature-major ([128, n_edges]) directly -- no on-chip
    transposes and no slow GPSIMD data movement.
  - dma_gather indices are int16 (< 32768), so nodes are split in two
    halves at 25000 and each core's edge shard is sorted host-side into 4
    segments by (src>=25000, dst>=25000); within a segment the gather
    source AP (x rows [0:25000) or [25000:50000)) is a compile-time
    constant and indices fit int16.
  - L1 is computed as three accumulated matmuls per 512-edge sub-tile
    (src K=128, dst K=128, attr K=16) into PSUM, relu+bias on the scalar
    engine (-> bf16), L2 as two accumulated matmuls, bias on the vector
    engine (-> bf16), and the transposed output tile is DMA'd to DRAM.
  - Host un-permutes/transposes per-core outputs into the final [E, 128].

The bass program is built after the inputs are seen (kernel compiles per
call); segment sizes are data-dependent but identical across cores (padded to
the max over cores).
"""

import numpy as np
import ml_dtypes

import concourse.bacc as bacc
import concourse.tile as tile
import concourse.mybir as mybir
from concourse import library_config
from concourse.bass_utils import run_bass_kernel_spmd

N_NODES = 50000
N_EDGES = 800000
C = 128      # in_channels
H = 256      # hidden
OUT = 128    # out_channels
HOP = 16
HALF = 25000  # node-id split so gather indices fit int16
N_CORES = 8
SUB = 512    # matmul sub-tile (edges)
NG = 4096    # gather tile (edges), multiple of SUB

BF16 = mybir.dt.bfloat16
F32 = mybir.dt.float32
I16 = mybir.dt.int16


def _compute_layout(edge_index):
    epc = N_EDGES // N_CORES
    src = edge_index[0].astype(np.int64)
    dst = edge_index[1].astype(np.int64)
    core_perms = []
    core_seg_counts = np.zeros((N_CORES, 4), np.int64)
    for c in range(N_CORES):
        lo, hi = c * epc, (c + 1) * epc
        s, d = src[lo:hi], dst[lo:hi]
        g = (s >= HALF) * 2 + (d >= HALF)
        order = np.argsort(g, kind="stable")
        core_perms.append(lo + order)
        core_seg_counts[c] = np.bincount(g, minlength=4)
    pad_sizes = [int(-(-core_seg_counts[:, g].max() // SUB) * SUB) for g in range(4)]
    return core_perms, core_seg_counts, pad_sizes


def _build_core_inputs(x, edge_index, edge_attr, W1, b1, W2, b2,
                       core_perms, core_seg_counts, pad_sizes):
    src = edge_index[0].astype(np.int64)
    dst = edge_index[1].astype(np.int64)
    E_pad = sum(pad_sizes)
    x_bf = np.ascontiguousarray(x.astype(ml_dtypes.bfloat16))  # [N, 128] node-major

    w1s = np.stack([np.ascontiguousarray(W1[hc * 128:(hc + 1) * 128, 0:C].T)
                    for hc in range(2)]).astype(ml_dtypes.bfloat16)
    w1d = np.stack([np.ascontiguousarray(W1[hc * 128:(hc + 1) * 128, C:2 * C].T)
                    for hc in range(2)]).astype(ml_dtypes.bfloat16)
    w1a = np.stack([np.ascontiguousarray(W1[hc * 128:(hc + 1) * 128, 2 * C:].T)
                    for hc in range(2)]).astype(ml_dtypes.bfloat16)
    w2 = np.stack([np.ascontiguousarray(W2[:, hc * 128:(hc + 1) * 128].T)
                   for hc in range(2)]).astype(ml_dtypes.bfloat16)
    b1c = np.stack([b1[hc * 128:(hc + 1) * 128].reshape(128, 1)
                    for hc in range(2)]).astype(np.float32)
    b2c = b2.reshape(128, 1).astype(np.float32)

    in_maps, row_maps = [], []
    for c in range(N_CORES):
        perm = core_perms[c]
        cnts = core_seg_counts[c]
        src16 = np.zeros(E_pad, np.int16)
        dst16 = np.zeros(E_pad, np.int16)
        attrT = np.zeros((HOP, E_pad), ml_dtypes.bfloat16)
        rows = np.full(E_pad, -1, np.int64)
        off_in = 0
        off_out = 0
        for g in range(4):
            n = int(cnts[g])
            ids = perm[off_in:off_in + n]
            sl = slice(off_out, off_out + n)
            s_off = HALF if (g >> 1) else 0
            d_off = HALF if (g & 1) else 0
            src16[sl] = (src[ids] - s_off).astype(np.int16)
            dst16[sl] = (dst[ids] - d_off).astype(np.int16)
            attrT[:, sl] = edge_attr[ids].T.astype(ml_dtypes.bfloat16)
            rows[sl] = ids
            off_in += n
            off_out += pad_sizes[g]

        def wrap(a):
            return np.ascontiguousarray(np.tile(a.reshape(-1, 16).T, (8, 1)))

        in_maps.append({
            "x": x_bf,
            "src_idx": wrap(src16),
            "dst_idx": wrap(dst16),
            "attrT": np.ascontiguousarray(attrT),
            "w1s": w1s, "w1d": w1d, "w1a": w1a, "w2": w2,
            "b1": b1c, "b2": b2c,
        })
        row_maps.append(rows)
    return in_maps, row_maps, E_pad


def _build_nc(pad_sizes, reps=1):
    E_pad = sum(pad_sizes)
    nc = bacc.Bacc("TRN2", target_bir_lowering=False, debug=False,
                   num_devices=N_CORES, dynamic_dma_scratch_size=1 << 15)
    x_d = nc.dram_tensor("x", [N_NODES, C], BF16, kind="ExternalInput")
    src_idx = nc.dram_tensor("src_idx", [128, E_pad // 16], I16, kind="ExternalInput")
    dst_idx = nc.dram_tensor("dst_idx", [128, E_pad // 16], I16, kind="ExternalInput")
    attrT = nc.dram_tensor("attrT", [HOP, E_pad], BF16, kind="ExternalInput")
    w1s_d = nc.dram_tensor("w1s", [2, C, 128], BF16, kind="ExternalInput")
    w1d_d = nc.dram_tensor("w1d", [2, C, 128], BF16, kind="ExternalInput")
    w1a_d = nc.dram_tensor("w1a", [2, HOP, 128], BF16, kind="ExternalInput")
    w2_d = nc.dram_tensor("w2", [2, 128, 128], BF16, kind="ExternalInput")
    b1_d = nc.dram_tensor("b1", [2, 128, 1], F32, kind="ExternalInput")
    b2_d = nc.dram_tensor("b2", [128, 1], F32, kind="ExternalInput")
    outT = nc.dram_tensor("outT", [128, E_pad], BF16, kind="ExternalOutput")

    with tile.TileContext(nc) as tc:
        import contextlib
        with contextlib.ExitStack() as ctx:
            consts = ctx.enter_context(tc.tile_pool(name="consts", bufs=1))
            idxp = ctx.enter_context(tc.tile_pool(name="idxp", bufs=1))
            gp = ctx.enter_context(tc.tile_pool(name="gp", bufs=2))
            ap_ = ctx.enter_context(tc.tile_pool(name="ap", bufs=2))
            hp = ctx.enter_context(tc.tile_pool(name="hp", bufs=4))
            op_ = ctx.enter_context(tc.tile_pool(name="op", bufs=3))
            ps1 = ctx.enter_context(tc.tile_pool(name="ps1", bufs=4, space="PSUM"))
            ps2p = ctx.enter_context(tc.tile_pool(name="ps2", bufs=2, space="PSUM"))

            nc.gpsimd.load_library(library_config.mlp)

            w1s_t = [consts.tile([C, 128], BF16, tag=f"w1s{i}", name=f"w1s{i}") for i in range(2)]
            w1d_t = [consts.tile([C, 128], BF16, tag=f"w1d{i}", name=f"w1d{i}") for i in range(2)]
            w1a_t = [consts.tile([HOP, 128], BF16, tag=f"w1a{i}", name=f"w1a{i}") for i in range(2)]
            w2_t = [consts.tile([128, 128], BF16, tag=f"w2{i}", name=f"w2{i}") for i in range(2)]
            b1_t = [consts.tile([128, 1], F32, tag=f"b1{i}", name=f"b1{i}") for i in range(2)]
            b2_t = consts.tile([128, 1], F32, tag="b2")
            for i in range(2):
                nc.sync.dma_start(w1s_t[i][:], w1s_d.ap()[i])
                nc.sync.dma_start(w1d_t[i][:], w1d_d.ap()[i])
                nc.sync.dma_start(w1a_t[i][:], w1a_d.ap()[i])
                nc.sync.dma_start(w2_t[i][:], w2_d.ap()[i])
                nc.sync.dma_start(b1_t[i][:], b1_d.ap()[i])
            nc.sync.dma_start(b2_t[:], b2_d.ap())

            sidx = idxp.tile([128, E_pad // 16], I16, tag="sidx")
            didx = idxp.tile([128, E_pad // 16], I16, tag="didx")
            nc.sync.dma_start(sidx[:], src_idx.ap())
            nc.sync.dma_start(didx[:], dst_idx.ap())

            x_lo = x_d.ap()[0:HALF]
            x_hi = x_d.ap()[HALF:N_NODES]

            Relu = mybir.ActivationFunctionType.Relu

            for _rep in range(reps):
                seg_start = 0
                for g in range(4):
                    npad = pad_sizes[g]
                    if npad == 0:
                        continue
                    src_tab = x_hi if (g >> 1) else x_lo
                    dst_tab = x_hi if (g & 1) else x_lo
                    e0 = seg_start
                    while e0 < seg_start + npad:
                        ng = min(NG, seg_start + npad - e0)
                        srcg = gp.tile([128, 1, NG], BF16, tag="srcg")
                        dstg = gp.tile([128, 1, NG], BF16, tag="dstg")
                        nc.gpsimd.dma_gather(
                            srcg[:, :, :ng], src_tab,
                            sidx[:, e0 // 16:(e0 + ng) // 16],
                            ng, ng, C, transpose=True)
                        nc.gpsimd.dma_gather(
                            dstg[:, :, :ng], dst_tab,
                            didx[:, e0 // 16:(e0 + ng) // 16],
                            ng, ng, C, transpose=True)
                        at = ap_.tile([HOP, NG], BF16, tag="at")
                        nc.sync.dma_start(at[:, :ng], attrT.ap()[:, e0:e0 + ng])
                        for st in range(ng // SUB):
                            col = slice(st * SUB, (st + 1) * SUB)
                            h_t = []
                            for hc in range(2):
                                ps = ps1.tile([128, SUB], F32, tag="ps1")
                                nc.tensor.matmul(ps[:], w1s_t[hc][:], srcg[:, 0, col],
                                                 start=True, stop=False)
                                nc.tensor.matmul(ps[:], w1d_t[hc][:], dstg[:, 0, col],
                                                 start=False, stop=False)
                                nc.tensor.matmul(ps[:], w1a_t[hc][:], at[:, col],
                                                 start=False, stop=True)
                                ht = hp.tile([128, SUB], BF16, tag="h")
                                nc.scalar.activation(ht[:], ps[:], Relu, bias=b1_t[hc][:])
                                h_t.append(ht)
                            ps2 = ps2p.tile([128, SUB], F32, tag="ps2")
                            nc.tensor.matmul(ps2[:], w2_t[0][:], h_t[0][:],
                                             start=True, stop=False)
                            nc.tensor.matmul(ps2[:], w2_t[1][:], h_t[1][:],
                                             start=False, stop=True)
                            ot = op_.tile([128, SUB], BF16, tag="ot")
                            nc.vector.tensor_scalar_add(ot[:], ps2[:], b2_t[:])
                            nc.sync.dma_start(
                                outT.ap()[:, e0 + st * SUB:e0 + (st + 1) * SUB], ot[:])
                        e0 += ng
                    seg_start += npad
    nc.compile()
    return nc


def _assemble_output(results, row_maps):
    out = np.empty((N_EDGES, OUT), np.float32)
    for c in range(N_CORES):
        rows = row_maps[c]
        m = rows >= 0
        out[rows[m]] = results[c]["outT"][:, m].T.astype(np.float32)
    return out


def build_all(x, edge_index, edge_attr, W1, b1, W2, b2, reps=1):
    """Build (nc, in_maps, row_maps) for the given inputs."""
    core_perms, core_seg_counts, pad_sizes = _compute_layout(edge_index)
    in_maps, row_maps, _ = _build_core_inputs(
        x, edge_index, edge_attr, W1, b1, W2, b2,
        core_perms, core_seg_counts, pad_sizes)
    nc = _build_nc(pad_sizes, reps=reps)
    return nc, in_maps, row_maps


def kernel(x, edge_index, edge_attr, W1, b1, W2, b2):
    x = np.asarray(x, np.float32)
    edge_index = np.asarray(edge_index)
    edge_attr = np.asarray(edge_attr, np.float32)
    W1 = np.asarray(W1, np.float32)
    b1 = np.asarray(b1, np.float32)
    W2 = np.asarray(W2, np.float32)
    b2 = np.asarray(b2, np.float32)
    assert x.shape == (N_NODES, C) and edge_index.shape == (2, N_EDGES)

    nc, in_maps, row_maps = build_all(x, edge_index, edge_attr, W1, b1, W2, b2)

    last_err = None
    for _attempt in range(3):
        try:
            res = run_bass_kernel_spmd(nc, in_maps, core_ids=list(range(N_CORES)))
            break
        except Exception as e:  # transient device errors: retry
            last_err = e
    else:
        raise last_err
    return _assemble_output(res.results, row_maps)


# revision 6
# speedup vs baseline: 252.2084x; 252.2084x over previous
"""DistanceConv2 GNN edge-MLP kernel for Trainium2 (8 NeuronCores).

out[e] = W2 @ relu(W1 @ [x[src_e]; x[dst_e]; attr_e] + b1) + b2   for 800K edges.

Strategy (edges sharded 8-way data-parallel, x + weights replicated):
  - x is stored node-major in DRAM (bf16 [50000, 128], 256B rows).
  - Endpoint features are fetched with gpsimd.dma_gather(transpose=True):
    GPSIMD only generates descriptors, the DMA engines move the data and
    write it feature-major ([128, n_edges]) directly -- no on-chip
    transposes and no slow GPSIMD data movement.
  - dma_gather indices are int16 (< 32768), so nodes are split in two
    halves at 25000 and each core's edge shard is sorted host-side into 4
    segments by (src>=25000, dst>=25000); within a segment the gather
    source AP (x rows [0:25000) or [25000:50000)) is a compile-time
    constant and indices fit int16.
  - L1 is computed as three accumulated matmuls per 512-edge sub-tile
    (src K=128, dst K=128, attr K=16) into PSUM, relu+bias on the scalar
    engine (-> bf16), L2 as two accumulated matmuls, bias on the vector
    engine (-> bf16), and the transposed output tile is DMA'd to DRAM.
  - Host un-permutes/transposes per-core outputs into the final [E, 128].

The bass program is built after the inputs are seen (kernel compiles per
call); segment sizes are data-dependent but identical across cores (padded to
the max over cores).
"""

import numpy as np
import ml_dtypes

import concourse.bacc as bacc
import concourse.tile as tile
import concourse.mybir as mybir
from concourse import library_config
from concourse.bass_utils import run_bass_kernel_spmd

N_NODES = 50000
N_EDGES = 800000
C = 128      # in_channels
H = 256      # hidden
OUT = 128    # out_channels
HOP = 16
HALF = 25000  # node-id split so gather indices fit int16
N_CORES = 8
NG = 896     # gather tile (edges); 58 SWDGE descs/direction fits the ring
SUB = 448    # matmul sub-tile (edges), NG == 2*SUB
NQ = 4       # SWDGE queues, gathers round-robin

BF16 = mybir.dt.bfloat16
F32 = mybir.dt.float32
I16 = mybir.dt.int16


def _compute_layout(edge_index):
    epc = N_EDGES // N_CORES
    src = edge_index[0].astype(np.int64)
    dst = edge_index[1].astype(np.int64)
    core_perms = []
    core_seg_counts = np.zeros((N_CORES, 4), np.int64)
    for c in range(N_CORES):
        lo, hi = c * epc, (c + 1) * epc
        s, d = src[lo:hi], dst[lo:hi]
        g = (s >= HALF) * 2 + (d >= HALF)
        order = np.argsort(g, kind="stable")
        core_perms.append(lo + order)
        core_seg_counts[c] = np.bincount(g, minlength=4)
    pad_sizes = [int(-(-core_seg_counts[:, g].max() // NG) * NG) for g in range(4)]
    return core_perms, core_seg_counts, pad_sizes


def _build_core_inputs(x, edge_index, edge_attr, W1, b1, W2, b2,
                       core_perms, core_seg_counts, pad_sizes):
    src = edge_index[0].astype(np.int64)
    dst = edge_index[1].astype(np.int64)
    E_pad = sum(pad_sizes)
    x_bf = np.ascontiguousarray(x.astype(ml_dtypes.bfloat16))  # [N, 128] node-major

    w1s = np.stack([np.ascontiguousarray(W1[hc * 128:(hc + 1) * 128, 0:C].T)
                    for hc in range(2)]).astype(ml_dtypes.bfloat16)
    w1d = np.stack([np.ascontiguousarray(W1[hc * 128:(hc + 1) * 128, C:2 * C].T)
                    for hc in range(2)]).astype(ml_dtypes.bfloat16)
    w1a = np.stack([np.ascontiguousarray(W1[hc * 128:(hc + 1) * 128, 2 * C:].T)
                    for hc in range(2)]).astype(ml_dtypes.bfloat16)
    w2 = np.stack([np.ascontiguousarray(W2[:, hc * 128:(hc + 1) * 128].T)
                   for hc in range(2)]).astype(ml_dtypes.bfloat16)
    b1c = np.stack([b1[hc * 128:(hc + 1) * 128].reshape(128, 1)
                    for hc in range(2)]).astype(np.float32)
    b2c = b2.reshape(128, 1).astype(np.float32)

    in_maps, row_maps = [], []
    for c in range(N_CORES):
        perm = core_perms[c]
        cnts = core_seg_counts[c]
        src16 = np.zeros(E_pad, np.int16)
        dst16 = np.zeros(E_pad, np.int16)
        attrT = np.zeros((HOP, E_pad), ml_dtypes.bfloat16)
        rows = np.full(E_pad, -1, np.int64)
        off_in = 0
        off_out = 0
        for g in range(4):
            n = int(cnts[g])
            ids = perm[off_in:off_in + n]
            sl = slice(off_out, off_out + n)
            s_off = HALF if (g >> 1) else 0
            d_off = HALF if (g & 1) else 0
            src16[sl] = (src[ids] - s_off).astype(np.int16)
            dst16[sl] = (dst[ids] - d_off).astype(np.int16)
            attrT[:, sl] = edge_attr[ids].T.astype(ml_dtypes.bfloat16)
            rows[sl] = ids
            off_in += n
            off_out += pad_sizes[g]

        def wrap(a):
            return np.ascontiguousarray(np.tile(a.reshape(-1, 16).T, (8, 1)))

        in_maps.append({
            "x": x_bf,
            "src_idx": wrap(src16),
            "dst_idx": wrap(dst16),
            "attrT": np.ascontiguousarray(attrT),
            "w1s": w1s, "w1d": w1d, "w1a": w1a, "w2": w2,
            "b1": b1c, "b2": b2c,
        })
        row_maps.append(rows)
    return in_maps, row_maps, E_pad


def _build_nc(pad_sizes, reps=1, num_devices=N_CORES):
    E_pad = sum(pad_sizes)
    nc = bacc.Bacc("TRN2", target_bir_lowering=False, debug=False,
                   num_devices=num_devices, num_swdge_queues=NQ)
    x_d = nc.dram_tensor("x", [N_NODES, C], BF16, kind="ExternalInput")
    src_idx = nc.dram_tensor("src_idx", [128, E_pad // 16], I16, kind="ExternalInput")
    dst_idx = nc.dram_tensor("dst_idx", [128, E_pad // 16], I16, kind="ExternalInput")
    attrT = nc.dram_tensor("attrT", [HOP, E_pad], BF16, kind="ExternalInput")
    w1s_d = nc.dram_tensor("w1s", [2, C, 128], BF16, kind="ExternalInput")
    w1d_d = nc.dram_tensor("w1d", [2, C, 128], BF16, kind="ExternalInput")
    w1a_d = nc.dram_tensor("w1a", [2, HOP, 128], BF16, kind="ExternalInput")
    w2_d = nc.dram_tensor("w2", [2, 128, 128], BF16, kind="ExternalInput")
    b1_d = nc.dram_tensor("b1", [2, 128, 1], F32, kind="ExternalInput")
    b2_d = nc.dram_tensor("b2", [128, 1], F32, kind="ExternalInput")
    outT = nc.dram_tensor("outT", [128, E_pad], BF16, kind="ExternalOutput")

    with tile.TileContext(nc) as tc:
        import contextlib
        with contextlib.ExitStack() as ctx:
            consts = ctx.enter_context(tc.tile_pool(name="consts", bufs=1))
            idxp = ctx.enter_context(tc.tile_pool(name="idxp", bufs=1))
            gp = ctx.enter_context(tc.tile_pool(name="gp", bufs=2))
            ap_ = ctx.enter_context(tc.tile_pool(name="ap", bufs=2))
            hp = ctx.enter_context(tc.tile_pool(name="hp", bufs=4))
            op_ = ctx.enter_context(tc.tile_pool(name="op", bufs=3))
            ps1 = ctx.enter_context(tc.tile_pool(name="ps1", bufs=4, space="PSUM"))
            ps2p = ctx.enter_context(tc.tile_pool(name="ps2", bufs=2, space="PSUM"))

            nc.gpsimd.load_library(library_config.mlp)

            w1s_t = [consts.tile([C, 128], BF16, tag=f"w1s{i}", name=f"w1s{i}") for i in range(2)]
            w1d_t = [consts.tile([C, 128], BF16, tag=f"w1d{i}", name=f"w1d{i}") for i in range(2)]
            w1a_t = [consts.tile([HOP, 128], BF16, tag=f"w1a{i}", name=f"w1a{i}") for i in range(2)]
            w2_t = [consts.tile([128, 128], BF16, tag=f"w2{i}", name=f"w2{i}") for i in range(2)]
            b1_t = [consts.tile([128, 1], F32, tag=f"b1{i}", name=f"b1{i}") for i in range(2)]
            b2_t = consts.tile([128, 1], F32, tag="b2")
            for i in range(2):
                nc.sync.dma_start(w1s_t[i][:], w1s_d.ap()[i])
                nc.sync.dma_start(w1d_t[i][:], w1d_d.ap()[i])
                nc.sync.dma_start(w1a_t[i][:], w1a_d.ap()[i])
                nc.sync.dma_start(w2_t[i][:], w2_d.ap()[i])
                nc.sync.dma_start(b1_t[i][:], b1_d.ap()[i])
            nc.sync.dma_start(b2_t[:], b2_d.ap())

            sidx = idxp.tile([128, E_pad // 16], I16, tag="sidx")
            didx = idxp.tile([128, E_pad // 16], I16, tag="didx")
            nc.sync.dma_start(sidx[:], src_idx.ap())
            nc.sync.dma_start(didx[:], dst_idx.ap())

            x_lo = x_d.ap()[0:HALF]
            x_hi = x_d.ap()[HALF:N_NODES]

            Relu = mybir.ActivationFunctionType.Relu
            qn = [0]

            def emit_rep():
                seg_start = 0
                for g in range(4):
                    npad = pad_sizes[g]
                    if npad == 0:
                        continue
                    src_tab = x_hi if (g >> 1) else x_lo
                    dst_tab = x_hi if (g & 1) else x_lo
                    e0 = seg_start
                    while e0 < seg_start + npad:
                        ng = NG
                        srcg = gp.tile([128, 1, NG], BF16, tag="srcg")
                        dstg = gp.tile([128, 1, NG], BF16, tag="dstg")
                        nc.gpsimd.dma_gather(
                            srcg[:, :, :ng], src_tab,
                            sidx[:, e0 // 16:(e0 + ng) // 16],
                            ng, ng, C, transpose=True, queue_num=qn[0] % NQ)
                        qn[0] += 1
                        nc.gpsimd.dma_gather(
                            dstg[:, :, :ng], dst_tab,
                            didx[:, e0 // 16:(e0 + ng) // 16],
                            ng, ng, C, transpose=True, queue_num=qn[0] % NQ)
                        qn[0] += 1
                        at = ap_.tile([HOP, NG], BF16, tag="at")
                        nc.sync.dma_start(at[:, :ng], attrT.ap()[:, e0:e0 + ng])
                        for st in range(ng // SUB):
                            col = slice(st * SUB, (st + 1) * SUB)
                            h_t = []
                            for hc in range(2):
                                ps = ps1.tile([128, SUB], F32, tag="ps1")
                                nc.tensor.matmul(ps[:], w1s_t[hc][:], srcg[:, 0, col],
                                                 start=True, stop=False)
                                nc.tensor.matmul(ps[:], w1d_t[hc][:], dstg[:, 0, col],
                                                 start=False, stop=False)
                                nc.tensor.matmul(ps[:], w1a_t[hc][:], at[:, col],
                                                 start=False, stop=True)
                                ht = hp.tile([128, SUB], BF16, tag="h")
                                nc.scalar.activation(ht[:], ps[:], Relu, bias=b1_t[hc][:])
                                h_t.append(ht)
                            ps2 = ps2p.tile([128, SUB], F32, tag="ps2")
                            nc.tensor.matmul(ps2[:], w2_t[0][:], h_t[0][:],
                                             start=True, stop=False)
                            nc.tensor.matmul(ps2[:], w2_t[1][:], h_t[1][:],
                                             start=False, stop=True)
                            ot = op_.tile([128, SUB], BF16, tag="ot")
                            nc.vector.tensor_scalar_add(ot[:], ps2[:], b2_t[:])
                            nc.sync.dma_start(
                                outT.ap()[:, e0 + st * SUB:e0 + (st + 1) * SUB], ot[:])
                        e0 += ng
                    seg_start += npad

            if reps == 1:
                emit_rep()
            else:
                with tc.For_i(0, reps):
                    emit_rep()
    nc.compile()
    return nc


def _assemble_output(results, row_maps):
    out = np.empty((N_EDGES, OUT), np.float32)
    for c in range(N_CORES):
        rows = row_maps[c]
        m = rows >= 0
        out[rows[m]] = results[c]["outT"][:, m].T.astype(np.float32)
    return out


def build_all(x, edge_index, edge_attr, W1, b1, W2, b2, reps=1):
    """Build (nc, in_maps, row_maps) for the given inputs."""
    core_perms, core_seg_counts, pad_sizes = _compute_layout(edge_index)
    in_maps, row_maps, _ = _build_core_inputs(
        x, edge_index, edge_attr, W1, b1, W2, b2,
        core_perms, core_seg_counts, pad_sizes)
    nc = _build_nc(pad_sizes, reps=reps)
    return nc, in_maps, row_maps


def kernel(x, edge_index, edge_attr, W1, b1, W2, b2):
    x = np.asarray(x, np.float32)
    edge_index = np.asarray(edge_index)
    edge_attr = np.asarray(edge_attr, np.float32)
    W1 = np.asarray(W1, np.float32)
    b1 = np.asarray(b1, np.float32)
    W2 = np.asarray(W2, np.float32)
    b2 = np.asarray(b2, np.float32)
    assert x.shape == (N_NODES, C) and edge_index.shape == (2, N_EDGES)

    nc, in_maps, row_maps = build_all(x, edge_index, edge_attr, W1, b1, W2, b2)

    last_err = None
    for _attempt in range(3):
        try:
            res = run_bass_kernel_spmd(nc, in_maps, core_ids=list(range(N_CORES)))
            break
        except Exception as e:  # transient device errors: retry
            last_err = e
    else:
        raise last_err
    return _assemble_output(res.results, row_maps)
